# revision 1
# baseline (speedup 1.0000x reference)
"""Trainium2 Bass kernel for a 2-layer CharRNN LSTM (B=64, T=256, V=128, H=1024).

Strategy: 8-way tensor-parallel over the gate dimension. Core i owns H-slice
[128*i, 128*(i+1)) and computes the 512 gate columns {i,f,o,g} for that slice
(column order [i|f|o|g] so one sigmoid covers cols 0:384 and one tanh covers
384:512). Per step, the recurrent matmul uses the *gathered* transposed hidden
state as the stationary operand (8 K-tiles of [128, 64]) and streams the
pre-sliced weight matrix [128, 512] as the moving operand. Each core then
computes its h-slice, transposes it on the TensorEngine, and the slices are
exchanged with a per-step AllGather (bf16, 16KB per rank per layer).

The two LSTM layers are interleaved: macro-step t runs layer0 step t,
layer1 step t-1, and the output head for step t-2, so each layer's AllGather
latency is hidden behind the other layer's compute. Layer1's input
contribution (y0 @ W_ih1^T) is fused into its PSUM accumulation using the
gathered layer0 h-tiles as stationary operands. Biases are accumulated into
PSUM via a K=1 matmul (ones ⊗ bias_row) that opens each accumulation group.

Matmul operands are bf16 (fp32 PSUM accumulation); the cell state c and all
elementwise math stay fp32.
"""
import numpy as np
import ml_dtypes

import concourse.bass as bass
import concourse.mybir as mybir
import concourse.tile as tile
from concourse import bacc, bass_utils
from concourse.masks import make_identity

N_CORES = 8
B = 64
V = 128
H = 1024
HSL = H // N_CORES  # 128, per-core H slice
GC = 4 * HSL        # 512, per-core gate columns
KT = H // 128       # 8 K-tiles

F32 = mybir.dt.float32
BF16 = mybir.dt.bfloat16
BF16_NP = ml_dtypes.bfloat16

last_result = None  # BassKernelResults of the most recent run (for test.py)
_build_cache = {}


def build(T):
    nc = bacc.Bacc("TRN2", target_bir_lowering=False, debug=False,
                   num_devices=N_CORES)

    # ---- DRAM I/O ----
    xT_d = nc.dram_tensor("xT", [V, T, B], BF16, kind="ExternalInput")
    whh0_d = nc.dram_tensor("whh0", [KT, 128, GC], BF16, kind="ExternalInput")
    wih0_d = nc.dram_tensor("wih0", [V, GC], BF16, kind="ExternalInput")
    whh1_d = nc.dram_tensor("whh1", [KT, 128, GC], BF16, kind="ExternalInput")
    wih1_d = nc.dram_tensor("wih1", [KT, 128, GC], BF16, kind="ExternalInput")
    wout_d = nc.dram_tensor("wout", [KT, 128, V], BF16, kind="ExternalInput")
    b0_d = nc.dram_tensor("b0", [1, GC], BF16, kind="ExternalInput")
    b1_d = nc.dram_tensor("b1", [1, GC], BF16, kind="ExternalInput")
    bout_d = nc.dram_tensor("bout", [1, V], BF16, kind="ExternalInput")
    hT0_d = nc.dram_tensor("hT0", [2, KT, 128, B], BF16, kind="ExternalInput")
    cinit_d = nc.dram_tensor("cinit", [2, B, HSL], F32, kind="ExternalInput")

    act_d = nc.dram_tensor("act", [B, T, V], F32, kind="ExternalOutput")
    h_d = nc.dram_tensor("h_out", [2, B, HSL], F32, kind="ExternalOutput")
    c_d = nc.dram_tensor("c_out", [2, B, HSL], F32, kind="ExternalOutput")

    with tile.TileContext(nc) as tc:
        with (
            tc.tile_pool(name="wp", bufs=1) as wp,          # persistent
            tc.tile_pool(name="sp", bufs=3) as sp,          # step temporaries
            tc.tile_pool(name="hp", bufs=2) as hp,          # gathered h tiles
            tc.tile_pool(name="pp", bufs=1, space="PSUM") as pp,
            tc.tile_pool(name="pg", bufs=2, space="PSUM") as pg,
            tc.tile_pool(name="dp", bufs=2, space="DRAM") as dp,
        ):
            # ---- persistent SBUF ----
            xT = wp.tile([V, T, B], BF16, name="xT_sb")
            whh0 = wp.tile([128, KT, GC], BF16, name="whh0_sb")
            wih0 = wp.tile([V, GC], BF16, name="wih0_sb")
            whh1 = wp.tile([128, KT, GC], BF16, name="whh1_sb")
            wih1 = wp.tile([128, KT, GC], BF16, name="wih1_sb")
            wout = wp.tile([128, KT, V], BF16, name="wout_sb")
            b0 = wp.tile([1, GC], BF16, name="b0_sb")
            b1 = wp.tile([1, GC], BF16, name="b1_sb")
            bout = wp.tile([1, V], BF16, name="bout_sb")
            ones = wp.tile([1, B], BF16, name="ones_sb")
            ident = wp.tile([B, B], F32, name="ident_sb")
            # [tanh(g) | c] per layer; c (cols HSL:2*HSL) persists across steps
            tgc = [wp.tile([B, 2 * HSL], F32, name=f"tgc{l}") for l in range(2)]

            nc.sync.dma_start(xT[:], xT_d[:])
            nc.sync.dma_start(whh0[:], whh0_d.ap().rearrange("k p n -> p k n"))
            nc.sync.dma_start(wih0[:], wih0_d[:])
            nc.sync.dma_start(whh1[:], whh1_d.ap().rearrange("k p n -> p k n"))
            nc.sync.dma_start(wih1[:], wih1_d.ap().rearrange("k p n -> p k n"))
            nc.sync.dma_start(wout[:], wout_d.ap().rearrange("k p n -> p k n"))
            nc.sync.dma_start(b0[:], b0_d[:])
            nc.sync.dma_start(b1[:], b1_d[:])
            nc.sync.dma_start(bout[:], bout_d[:])
            for l in range(2):
                nc.sync.dma_start(tgc[l][:, HSL:], cinit_d[l])
            nc.vector.memset(ones[:], 1.0)
            make_identity(nc, ident[:])

            hT_cur = [None, None]  # gathered [128, KT, B] bf16 per layer
            for l in range(2):
                t0 = hp.tile([128, KT, B], BF16, name=f"hTinit{l}", tag=f"hT{l}")
                nc.sync.dma_start(t0[:], hT0_d[l].rearrange("k p n -> p k n"))
                hT_cur[l] = t0

            whh = [whh0, whh1]
            bias = [b0, b1]
            h_last = [None, None]  # final h slices (untransposed, fp32)

            def lstm_elem_and_gather(l, g, tag):
                """Gate PSUM [B, GC] -> h slice, c update, transpose, AllGather.

                Returns (h_tile, new gathered hT tile)."""
                sig = sp.tile([B, 3 * HSL], F32, name=f"sig_{tag}", tag=f"sig{l}")
                nc.scalar.activation(tgc[l][:, 0:HSL], g[:, 3 * HSL:],
                                     mybir.ActivationFunctionType.Tanh)
                nc.scalar.activation(sig[:], g[:, 0:3 * HSL],
                                     mybir.ActivationFunctionType.Sigmoid)
                t12 = sp.tile([B, 2 * HSL], F32, name=f"t12_{tag}", tag=f"t12{l}")
                nc.vector.tensor_mul(t12[:], sig[:, 0:2 * HSL], tgc[l][:])
                nc.vector.tensor_add(tgc[l][:, HSL:], t12[:, 0:HSL], t12[:, HSL:])
                tc_ = sp.tile([B, HSL], F32, name=f"tc_{tag}", tag=f"tc{l}")
                nc.scalar.activation(tc_[:], tgc[l][:, HSL:],
                                     mybir.ActivationFunctionType.Tanh)
                h = sp.tile([B, HSL], F32, name=f"h_{tag}", tag=f"h{l}")
                nc.vector.tensor_mul(h[:], sig[:, 2 * HSL:], tc_[:])
                # transpose -> bf16 -> bounce -> AllGather -> gathered tile
                tr = pp.tile([HSL, B], F32, name=f"tr_{tag}", tag=f"tr{l}")
                nc.tensor.transpose(tr[:], h[:], ident[:])
                snd = sp.tile([HSL, B], BF16, name=f"snd_{tag}", tag=f"snd{l}")
                nc.vector.tensor_copy(snd[:], tr[:])
                cin = dp.tile([HSL, B], BF16, name=f"cin_{tag}", tag=f"cin{l}")
                cout = dp.tile([HSL * N_CORES, B], BF16, name=f"cout_{tag}",
                               tag=f"cout{l}", addr_space="Shared")
                nc.sync.dma_start(cin[:], snd[:])
                nc.gpsimd.collective_compute(
                    "AllGather", mybir.AluOpType.bypass,
                    replica_groups=[list(range(N_CORES))],
                    ins=[cin.opt()], outs=[cout.opt()])
                gath = hp.tile([128, KT, B], BF16, name=f"hT_{tag}", tag=f"hT{l}")
                nc.sync.dma_start(gath[:],
                                  cout[:].rearrange("(k p) n -> p k n", k=KT))
                return h, gath

            for t in range(T + 2):
                if t < T:
                    # ---- layer 0, step t ----
                    g0 = pg.tile([B, GC], F32, name=f"g0_{t}", tag="g0")
                    nc.tensor.matmul(g0[:], ones[:], b0[:], start=True, stop=False)
                    for k in range(KT):
                        nc.tensor.matmul(g0[:], hT_cur[0][:, k, :], whh0[:, k, :],
                                         start=False, stop=False)
                    nc.tensor.matmul(g0[:], xT[:, t, :], wih0[:],
                                     start=False, stop=True)
                    h0_new, hT0_new = lstm_elem_and_gather(0, g0, f"l0_{t}")
                if 1 <= t <= T:
                    # ---- layer 1, step t-1 (uses hT_cur[0] = gathered y0(t-1)) ----
                    g1 = pg.tile([B, GC], F32, name=f"g1_{t}", tag="g1")
                    nc.tensor.matmul(g1[:], ones[:], b1[:], start=True, stop=False)
                    for k in range(KT):
                        nc.tensor.matmul(g1[:], hT_cur[1][:, k, :], whh1[:, k, :],
                                         start=False, stop=False)
                    for k in range(KT):
                        nc.tensor.matmul(g1[:], hT_cur[0][:, k, :], wih1[:, k, :],
                                         start=False, stop=(k == KT - 1))
                    h1_new, hT1_new = lstm_elem_and_gather(1, g1, f"l1_{t}")
                if t >= 2:
                    # ---- head, step t-2 (uses hT_cur[1] = gathered y1(t-2)) ----
                    hd = pp.tile([B, V], F32, name=f"hd_{t}", tag="hd")
                    nc.tensor.matmul(hd[:], ones[:], bout[:], start=True, stop=False)
                    for k in range(KT):
                        nc.tensor.matmul(hd[:], hT_cur[1][:, k, :], wout[:, k, :],
                                         start=False, stop=(k == KT - 1))
                    a = sp.tile([B, V], F32, name=f"a_{t}", tag="a")
                    nc.scalar.copy(a[:], hd[:])
                    nc.sync.dma_start(act_d[:, t - 2, :], a[:])
                # rotate gathered-tile refs; keep final h slices
                if t < T:
                    hT_cur[0] = hT0_new
                    if t == T - 1:
                        h_last[0] = h0_new
                if 1 <= t <= T:
                    hT_cur[1] = hT1_new
                    if t == T:
                        h_last[1] = h1_new

            for l in range(2):
                nc.sync.dma_start(h_d[l], h_last[l][:])
                nc.sync.dma_start(c_d[l], tgc[l][:, HSL:])

    nc.compile()
    return nc


def _gate_rows(i):
    """Row indices into the 4H gate dim for core i, in [i|f|o|g] order."""
    hs = np.arange(HSL * i, HSL * (i + 1))
    return np.concatenate([0 * H + hs, 1 * H + hs, 3 * H + hs, 2 * H + hs])


def kernel(x, h0, c0, W_ih0, W_hh0, b0, W_ih1, W_hh1, b1, W_out, b_out):
    global last_result
    x = np.asarray(x, np.float32)
    h0 = np.asarray(h0, np.float32)
    c0 = np.asarray(c0, np.float32)
    Bx, T, Vx = x.shape
    assert (Bx, Vx) == (B, V), (x.shape,)

    if T not in _build_cache:
        _build_cache[T] = build(T)
    nc = _build_cache[T]

    xT = np.ascontiguousarray(x.transpose(2, 1, 0)).astype(BF16_NP)  # [V,T,B]
    hT0 = np.stack([
        np.asarray(h0[l]).T.reshape(KT, 128, B) for l in range(2)
    ]).astype(BF16_NP)  # [2,KT,128,B]
    woutT = np.ascontiguousarray(np.asarray(W_out, np.float32).T)  # [H, V]
    woutT = woutT.reshape(KT, 128, V).astype(BF16_NP)
    boutr = np.asarray(b_out, np.float32)[None, :].astype(BF16_NP)

    in_maps = []
    for i in range(N_CORES):
        rows = _gate_rows(i)
        hs = slice(HSL * i, HSL * (i + 1))
        wih0s = np.ascontiguousarray(np.asarray(W_ih0, np.float32)[rows].T)
        whh0s = np.ascontiguousarray(np.asarray(W_hh0, np.float32)[rows].T)
        wih1s = np.ascontiguousarray(np.asarray(W_ih1, np.float32)[rows].T)
        whh1s = np.ascontiguousarray(np.asarray(W_hh1, np.float32)[rows].T)
        in_maps.append({
            "xT": xT,
            "whh0": whh0s.reshape(KT, 128, GC).astype(BF16_NP),
            "wih0": wih0s.astype(BF16_NP),  # [V, GC]
            "whh1": whh1s.reshape(KT, 128, GC).astype(BF16_NP),
            "wih1": wih1s.reshape(KT, 128, GC).astype(BF16_NP),
            "wout": woutT,
            "b0": np.asarray(b0, np.float32)[rows][None, :].astype(BF16_NP),
            "b1": np.asarray(b1, np.float32)[rows][None, :].astype(BF16_NP),
            "bout": boutr,
            "hT0": hT0,
            "cinit": np.ascontiguousarray(np.asarray(c0, np.float32)[:, :, hs]),
        })

    last_result = bass_utils.run_bass_kernel_spmd(
        nc, in_maps, core_ids=list(range(N_CORES)))
    res = last_result.results

    act = np.asarray(res[0]["act"], np.float32).reshape(B * T, V)
    h = np.zeros((2, B, H), np.float32)
    c = np.zeros((2, B, H), np.float32)
    for i in range(N_CORES):
        hs = slice(HSL * i, HSL * (i + 1))
        h[:, :, hs] = res[i]["h_out"]
        c[:, :, hs] = res[i]["c_out"]
    return act, h, c
